# revision 12
# baseline (speedup 1.0000x reference)
"""Multi-head attention (B=2, T=2048, D=1024, H=16, no causal mask) on 8 trn2
NeuronCores.

Sharding: (batch x head-quad).  Core c handles batch b = c // 4 and heads
[4*hq, 4*hq+4) with hq = c % 4.  Each core computes Q/K/V for only its 4
heads over the full 2048 tokens of its batch (no redundant projection work),
runs full attention for those heads, and emits a PARTIAL o-proj
y_c = out_heads @ Wo[head_rows] as fp16.  The host sums the 4 partials per
batch (the unshard step) - no on-device collective (collective_compute has an
~80us floor in this environment).

Host-side prep: X[b] is transposed on the host so the kernel runs zero PE
transposes; weight slices are pre-cut per core.

Per-core schedule (pair p holds heads 2p/2p+1 on partitions 0-63 / 64-127
of qt/kt row chunks; ScalarE exp at ~1.1us per [128,1024] tile is the pacing
engine at ~142us total):
  prelude: KT/QT for pair 0 (kd-outer, 2 open accumulators to amortize
           LDWEIGHTS), then V chunks in token order.
  attention (pair-outer, query-block inner, 16 key chunks):
      lg[k, q] = KT_h^T @ QT_h   (PSUM [128, 2*512]; hh row-tiles run
                                  concurrently on the PE)
      PT = exp(0.125 * lg)       (ScalarE -> fp16 SBUF)
      PV(kc-1) is emitted AFTER logits(kc) so the in-order TensorE stream
      never blocks waiting on exp; KT/QT for pair 1 are drip-fed into the
      PE slack during pair 0's attention, o-proj for block qb into pair 1's
      (qb+1) stream.
  o-proj: y_partial[q,:] = ot^T @ Wo_p accumulated over both pairs.
"""

import numpy as np

import concourse.bacc as bacc
import concourse.mybir as mybir
import concourse.tile as tile

F32 = mybir.dt.float32
F16 = mybir.dt.float16

B, T, D, H = 2, 2048, 1024, 16
DH = D // H          # 64
P = 128
N_CORES = 8
NH = 4               # heads per core
NPAIR = 2            # head pairs per core
KD = D // P          # 8 contraction chunks over D
NT = T // P          # 16 key-token chunks
TQ = 512             # query block
NQ = T // TQ         # 4 query blocks
VW = DH + 1          # 65: V head slot width incl. ones column
EXPF = mybir.ActivationFunctionType.Exp


def build_nc():
    nc = bacc.Bacc("TRN2", target_bir_lowering=False, debug=False,
                   num_devices=N_CORES)
    xt_in = nc.dram_tensor("xt_in", [D, T], F16, kind="ExternalInput").ap()
    wq = nc.dram_tensor("wq", [D, NH * DH], F16, kind="ExternalInput").ap()
    wk = nc.dram_tensor("wk", [D, NH * DH], F16, kind="ExternalInput").ap()
    wv = nc.dram_tensor("wv", [D, NH * DH], F16, kind="ExternalInput").ap()
    wo = nc.dram_tensor("wo", [NH * DH, D], F16, kind="ExternalInput").ap()
    yp = nc.dram_tensor("yp", [T, D], F16, kind="ExternalOutput").ap()

    wq3 = wq.rearrange("(kd p) c -> p kd c", p=P)
    wk3 = wk.rearrange("(kd p) c -> p kd c", p=P)
    wv3 = wv.rearrange("(kd p) c -> p kd c", p=P)
    xt3 = xt_in.rearrange("(kd p) t -> p kd t", p=P)

    with tile.TileContext(nc) as tc:
      with tc.tile_pool(name="persist", bufs=1) as persist:
        xt = persist.tile([P, KD * T], F16)       # 32 KB/part
        qt = persist.tile([P, NPAIR * T], F16)    # 8 KB/part
        kt = persist.tile([P, NPAIR * T], F16)    # 8 KB/part
        v_sb = persist.tile([P, NT * NH * VW], F16)  # 8.1 KB/part
        wqs = persist.tile([P, KD * 256], F16)    # 4 KB/part
        wks = persist.tile([P, KD * 256], F16)
        wvs = persist.tile([P, KD * 256], F16)
        wos = persist.tile([P, NPAIR * D], F16)   # 4 KB/part
        ot = persist.tile([P, NPAIR * T], F16)    # 8 KB/part

        # weights first (small), then xt in kd chunks so matmuls can start
        # before the whole 4 MB lands
        nc.sync.dma_start(wqs.rearrange("p (kd c) -> p kd c", c=256), wq3)
        nc.sync.dma_start(wks.rearrange("p (kd c) -> p kd c", c=256), wk3)
        nc.sync.dma_start(wvs.rearrange("p (kd c) -> p kd c", c=256), wv3)
        nc.sync.dma_start(wos.rearrange("p (pp c) -> p pp c", c=D),
                          wo.rearrange("(pp r) c -> r pp c", r=P))
        for kd in range(KD):
            nc.sync.dma_start(xt[:, kd * T:(kd + 1) * T], xt3[:, kd])

        onec = persist.tile([P, 1], F16)
        nc.vector.memset(onec[:], 1.0)
        nc.vector.tensor_copy(
            v_sb.rearrange("p (s c) -> p s c", c=VW)[:, :, DH:DH + 1],
            onec.unsqueeze(1).broadcast_to((P, NT * NH, 1)))

        with (
            tc.tile_pool(name="auxps", bufs=2, space="PSUM") as auxps,
            tc.tile_pool(name="lgps", bufs=2, space="PSUM") as lgps,
            tc.tile_pool(name="pvps", bufs=2, space="PSUM") as pvps,
            tc.tile_pool(name="ptp", bufs=4) as ptp,
            tc.tile_pool(name="rcp", bufs=2) as rcp,
            tc.tile_pool(name="rbp", bufs=3) as rbp,
            tc.tile_pool(name="ysp", bufs=3) as ysp,
        ):
            # ---- projection emitters (used for prelude AND drip-feed) ----
            def emit_proj_halfpair(ws, dst, p, tbs):
                """KT or QT row chunk p for query blocks tbs (kd-outer,
                len(tbs) open accumulators)."""
                for u in proj_units(ws, dst, p, tbs):
                    u()

            def proj_units(ws, dst, p, tbs):
                """Split a halfpair projection into KD per-kd closures (the
                last also copies out) so it can drip into the PE stream."""
                acc = {}

                def unit(kd):
                    def run():
                        if kd == 0:
                            acc['t'] = [auxps.tile(
                                [P, TQ], F32, tag="aux",
                                name=f"pj_{id(ws)}_{p}_{tb}") for tb in tbs]
                        for i, tb in enumerate(tbs):
                            nc.tensor.matmul(
                                acc['t'][i][:],
                                ws[:, kd * 256 + p * P: kd * 256 + (p + 1) * P],
                                xt[:, kd * T + tb * TQ: kd * T + (tb + 1) * TQ],
                                start=(kd == 0), stop=(kd == KD - 1))
                        if kd == KD - 1:
                            for i, tb in enumerate(tbs):
                                nc.vector.tensor_copy(
                                    dst[:, p * T + tb * TQ:
                                        p * T + (tb + 1) * TQ],
                                    acc['t'][i][:])
                    return run
                return [unit(kd) for kd in range(KD)]

            def emit_v_chunk(tci):
                pv = auxps.tile([P, 256], F32, tag="aux", name=f"pv_{tci}")
                for kd in range(KD):
                    nc.tensor.matmul(
                        pv[:],
                        xt[:, kd * T + tci * P: kd * T + (tci + 1) * P],
                        wvs[:, kd * 256:(kd + 1) * 256],
                        start=(kd == 0), stop=(kd == KD - 1))
                dst = v_sb[:, tci * (NH * VW): (tci + 1) * (NH * VW)]
                nc.vector.tensor_copy(
                    dst.rearrange("p (h c) -> p h c", c=VW)[:, :, 0:DH],
                    pv.rearrange("p (h c) -> p h c", c=DH))

            # ---- prelude: pair 0 KT/QT, then V chunks ----
            emit_proj_halfpair(wks, kt, 0, [0, 1])
            emit_proj_halfpair(wks, kt, 0, [2, 3])
            emit_proj_halfpair(wqs, qt, 0, [0, 1])
            emit_proj_halfpair(wqs, qt, 0, [2, 3])
            for tci in range(NT):
                emit_v_chunk(tci)

            # drip-feed queue for pair-1 projections, consumed one unit per
            # key chunk during the attention stream of pair 0
            drip = []
            for args in ((wks, kt, 1, [0, 1]), (wks, kt, 1, [2, 3]),
                         (wqs, qt, 1, [0, 1]), (wqs, qt, 1, [2, 3])):
                drip.extend(proj_units(*args))

            def oproj_unit(qb, tc_):
                def run():
                    pys = [auxps.tile([P, TQ], F32, tag="aux",
                                      name=f"py{qb}_{tc_}_{n}")
                           for n in range(2)]
                    for p in range(NPAIR):
                        for nh in range(2):
                            nc.tensor.matmul(
                                pys[nh][:],
                                ot[:, p * T + qb * TQ + tc_ * P:
                                   p * T + qb * TQ + (tc_ + 1) * P],
                                wos[:, p * D + nh * TQ: p * D + (nh + 1) * TQ],
                                start=(p == 0), stop=(p == NPAIR - 1))
                    for nh in range(2):
                        ys = ysp.tile([P, TQ], F16, tag="ys")
                        nc.vector.tensor_copy(ys[:], pys[nh][:])
                        nc.sync.dma_start(
                            yp[qb * TQ + tc_ * P: qb * TQ + (tc_ + 1) * P,
                               nh * TQ:(nh + 1) * TQ],
                            ys[:])
                return run

            def normalize(p, qb, pva, pvb):
                for hi, pv_ in ((0, pva), (1, pvb)):
                    pvs = rcp.tile([VW, TQ], F32, tag="pvs")
                    nc.vector.tensor_copy(pvs[:], pv_[:])
                    sr = rcp.tile([1, TQ], F32, tag="sr")
                    nc.vector.tensor_copy(sr[0:1, :], pvs[DH:DH + 1, :])
                    rc = rcp.tile([1, TQ], F32, tag="rc")
                    nc.vector.reciprocal_approx_fast(rc[0:1, :], sr[0:1, :])
                    rb = rbp.tile([P, TQ], F32, tag="rb")
                    nc.sync.dma_start(
                        rb[0:DH, :],
                        rc[0:1, :].unsqueeze(1).broadcast_to((1, DH, TQ)))
                    if hi == 0:
                        nc.vector.tensor_mul(
                            ot[0:DH, p * T + qb * TQ: p * T + (qb + 1) * TQ],
                            pvs[0:DH, :], rb[0:DH, :])
                    else:
                        sh = rbp.tile([P, TQ], F16, tag="sh")
                        nc.vector.tensor_mul(
                            sh[0:DH, :], pvs[0:DH, :], rb[0:DH, :])
                        nc.sync.dma_start(
                            ot[DH:P, p * T + qb * TQ: p * T + (qb + 1) * TQ],
                            sh[0:DH, :])

            def emit_pv(p, kc, pt_, pva, pvb):
                for hh, pv_ in ((0, pva), (1, pvb)):
                    h = 2 * p + hh
                    va = v_sb[:, kc * (NH * VW) + h * VW:
                              kc * (NH * VW) + h * VW + VW]
                    nc.tensor.matmul(
                        pv_[:], va, pt_[:, hh * TQ:(hh + 1) * TQ],
                        start=(kc == 0), stop=(kc == NT - 1))

            # ---- attention: pair-outer, query-block inner ----
            for p in range(NPAIR):
                for qb in range(NQ):
                    pva = pvps.tile([VW, TQ], F32, tag="pv")
                    pvb = pvps.tile([VW, TQ], F32, tag="pv")
                    pend = None  # PV deferred by one kc
                    for kc in range(NT):
                        lg = lgps.tile([P, 2 * TQ], F32, tag="lg")
                        for hh in range(2):
                            nc.tensor.matmul(
                                lg[:, hh * TQ:(hh + 1) * TQ],
                                kt[hh * DH:(hh + 1) * DH,
                                   p * T + kc * P: p * T + (kc + 1) * P],
                                qt[hh * DH:(hh + 1) * DH,
                                   p * T + qb * TQ: p * T + (qb + 1) * TQ],
                                start=True, stop=True)
                        # drip one queued unit (pair-1 projection kd-step or
                        # deferred o-proj token chunk) into the PE stream
                        if 1 <= kc <= 14 and drip:
                            drip.pop(0)()
                        pt_ = ptp.tile([P, 2 * TQ], F16, tag="pt")
                        nc.scalar.activation(pt_[:], lg[:], EXPF, scale=0.125)
                        if pend is not None:
                            emit_pv(p, pend[0], pend[1], pva, pvb)
                        pend = (kc, pt_)
                    emit_pv(p, pend[0], pend[1], pva, pvb)
                    normalize(p, qb, pva, pvb)
                    if p == 1:
                        # o-proj for this block drips into the next block's
                        # key-chunk stream; the last block's runs at the tail
                        drip.extend(oproj_unit(qb, tc_) for tc_ in range(NQ))
            while drip:
                drip.pop(0)()

    nc.compile()
    return nc


_NC_CACHE = None


def _get_nc():
    global _NC_CACHE
    if _NC_CACHE is None:
        _NC_CACHE = build_nc()
    return _NC_CACHE


def _shard_inputs(x, Wqkv, Wo):
    x16 = np.asarray(x, dtype=np.float32).astype(np.float16)
    w16 = np.asarray(Wqkv, dtype=np.float32).astype(np.float16)
    wo16 = np.asarray(Wo, dtype=np.float32).astype(np.float16)
    xts = [np.ascontiguousarray(x16[b].T) for b in range(B)]
    in_maps = []
    for c in range(N_CORES):
        b, hq = c // 4, c % 4
        cs = hq * NH * DH
        in_maps.append({
            "xt_in": xts[b],
            "wq": np.ascontiguousarray(w16[:, cs:cs + NH * DH]),
            "wk": np.ascontiguousarray(w16[:, D + cs: D + cs + NH * DH]),
            "wv": np.ascontiguousarray(w16[:, 2 * D + cs: 2 * D + cs + NH * DH]),
            "wo": np.ascontiguousarray(wo16[cs:cs + NH * DH, :]),
        })
    return in_maps


def kernel(x, Wqkv, Wo):
    from concourse.bass_utils import run_bass_kernel_spmd

    nc = _get_nc()
    in_maps = _shard_inputs(x, Wqkv, Wo)
    res = run_bass_kernel_spmd(nc, in_maps, core_ids=list(range(N_CORES)))
    out = np.zeros((B, T, D), dtype=np.float32)
    for c in range(N_CORES):
        b = c // 4
        out[b] += res.results[c]["yp"].astype(np.float32)
    return out


# revision 17
# speedup vs baseline: 1.1512x; 1.1512x over previous
"""Multi-head attention (B=2, T=2048, D=1024, H=16, no causal mask) on 8 trn2
NeuronCores.

Sharding: (batch x head-quad).  Core c handles batch b = c // 4 and heads
[4*hq, 4*hq+4) with hq = c % 4.  Each core computes Q/K/V for only its 4
heads over the full 2048 tokens of its batch (no redundant projection work),
runs full attention for those heads, and emits a PARTIAL o-proj
y_c = out_heads @ Wo[head_rows] as fp16.  The host sums the 4 partials per
batch (the unshard step) - no on-device collective (collective_compute has an
~80us floor in this environment).

Host-side prep: X[b] is transposed on the host so the kernel runs zero PE
transposes; weight slices are pre-cut per core.

Per-core schedule (pair p holds heads 2p/2p+1 on partitions 0-63 / 64-127
of qt/kt row chunks; ScalarE exp at ~1.1us per [128,1024] tile is the pacing
engine at ~142us total):
  prelude: KT/QT for pair 0 (kd-outer, 2 open accumulators to amortize
           LDWEIGHTS), then V chunks in token order.
  attention (pair-outer, query-block inner, 16 key chunks):
      lg[k, q] = KT_h^T @ QT_h   (PSUM [128, 2*512]; hh row-tiles run
                                  concurrently on the PE)
      PT = exp(0.125 * lg)       (ScalarE -> fp16 SBUF)
      PV(kc-1) is emitted AFTER logits(kc) so the in-order TensorE stream
      never blocks waiting on exp; KT/QT for pair 1 are drip-fed into the
      PE slack during pair 0's attention, o-proj for block qb into pair 1's
      (qb+1) stream.
  o-proj: y_partial[q,:] = ot^T @ Wo_p accumulated over both pairs.
"""

import numpy as np

import concourse.bacc as bacc
import concourse.mybir as mybir
import concourse.tile as tile

F32 = mybir.dt.float32
F16 = mybir.dt.float16

B, T, D, H = 2, 2048, 1024, 16
DH = D // H          # 64
P = 128
N_CORES = 8
NH = 4               # heads per core
NPAIR = 2            # head pairs per core
KD = D // P          # 8 contraction chunks over D
NT = T // P          # 16 key-token chunks
TQ = 512             # query block
NQ = T // TQ         # 4 query blocks
VW = DH + 1          # 65: V head slot width incl. ones column
VSLOT = NH * VW      # 260 per key chunk
EXPF = mybir.ActivationFunctionType.Exp


def build_nc():
    nc = bacc.Bacc("TRN2", target_bir_lowering=False, debug=False,
                   num_devices=N_CORES)
    xt_in = nc.dram_tensor("xt_in", [D, T], F16, kind="ExternalInput").ap()
    wq = nc.dram_tensor("wq", [D, NH * DH], F16, kind="ExternalInput").ap()
    wk = nc.dram_tensor("wk", [D, NH * DH], F16, kind="ExternalInput").ap()
    wv = nc.dram_tensor("wv", [D, NH * DH], F16, kind="ExternalInput").ap()
    wo = nc.dram_tensor("wo", [NH * DH, D], F16, kind="ExternalInput").ap()
    yp = nc.dram_tensor("yp", [T, D], F16, kind="ExternalOutput").ap()

    wq3 = wq.rearrange("(kd p) c -> p kd c", p=P)
    wk3 = wk.rearrange("(kd p) c -> p kd c", p=P)
    wv3 = wv.rearrange("(kd p) c -> p kd c", p=P)
    xt3 = xt_in.rearrange("(kd p) t -> p kd t", p=P)

    with tile.TileContext(nc) as tc:
      with tc.tile_pool(name="persist", bufs=1) as persist:
        xt = persist.tile([P, KD * T], F16)       # 32 KB/part
        qt = persist.tile([P, NPAIR * T], F16)    # 8 KB/part
        kt = persist.tile([P, NPAIR * T], F16)    # 8 KB/part
        v_sb = persist.tile([P, NT * VSLOT], F16)  # 8.1 KB/part
        wqs = persist.tile([P, KD * 256], F16)    # 4 KB/part
        wks = persist.tile([P, KD * 256], F16)
        wvs = persist.tile([P, KD * 256], F16)
        wos = persist.tile([P, NPAIR * D], F16)   # 4 KB/part
        ot = persist.tile([P, NPAIR * T], F16)    # 8 KB/part

        # weights first (small), then xt in kd chunks so matmuls can start
        # before the whole 4 MB lands
        nc.sync.dma_start(wqs.rearrange("p (kd c) -> p kd c", c=256), wq3)
        nc.sync.dma_start(wks.rearrange("p (kd c) -> p kd c", c=256), wk3)
        nc.sync.dma_start(wvs.rearrange("p (kd c) -> p kd c", c=256), wv3)
        nc.sync.dma_start(wos.rearrange("p (pp c) -> p pp c", c=D),
                          wo.rearrange("(pp r) c -> r pp c", r=P))
        for kd in range(KD):
            nc.sync.dma_start(xt[:, kd * T:(kd + 1) * T], xt3[:, kd])

        onec = persist.tile([P, 1], F16)
        nc.vector.memset(onec[:], 1.0)
        nc.vector.tensor_copy(
            v_sb.rearrange("p (s c) -> p s c", c=VW)[:, :, DH:DH + 1],
            onec.unsqueeze(1).broadcast_to((P, NT * NH, 1)))

        with tc.tile_pool(name="preps", bufs=8, space="PSUM") as preps:
            accs = {}
            for hp, (ws, dst, tbs) in enumerate(
                    ((wks, kt, (0, 1)), (wks, kt, (2, 3)),
                     (wqs, qt, (0, 1)), (wqs, qt, (2, 3)))):
                accs[hp] = [preps.tile([P, TQ], F32, tag="pre",
                                       name=f"pre_{hp}_{tb}") for tb in tbs]
            for kd in range(KD):
                for hp, (ws, dst, tbs) in enumerate(
                        ((wks, kt, (0, 1)), (wks, kt, (2, 3)),
                         (wqs, qt, (0, 1)), (wqs, qt, (2, 3)))):
                    for i, tb in enumerate(tbs):
                        nc.tensor.matmul(
                            accs[hp][i][:],
                            ws[:, kd * 256: kd * 256 + P],
                            xt[:, kd * T + tb * TQ: kd * T + (tb + 1) * TQ],
                            start=(kd == 0), stop=(kd == KD - 1))
            for hp, (ws, dst, tbs) in enumerate(
                    ((wks, kt, (0, 1)), (wks, kt, (2, 3)),
                     (wqs, qt, (0, 1)), (wqs, qt, (2, 3)))):
                for i, tb in enumerate(tbs):
                    nc.vector.tensor_copy(
                        dst[:, tb * TQ:(tb + 1) * TQ], accs[hp][i][:])

        with (
            tc.tile_pool(name="auxps", bufs=2, space="PSUM") as auxps,
            tc.tile_pool(name="lgps", bufs=2, space="PSUM") as lgps,
            tc.tile_pool(name="pvps", bufs=2, space="PSUM") as pvps,
            tc.tile_pool(name="ptp", bufs=4) as ptp,
            tc.tile_pool(name="rcp", bufs=2) as rcp,
            tc.tile_pool(name="rbp", bufs=3) as rbp,
            tc.tile_pool(name="ysp", bufs=3) as ysp,
        ):
            # ---- projection emitters (used for prelude AND drip-feed) ----
            def emit_proj_halfpair(ws, dst, p, tbs):
                """KT or QT row chunk p for query blocks tbs (kd-outer,
                len(tbs) open accumulators)."""
                for u in proj_units(ws, dst, p, tbs):
                    u()

            def proj_units(ws, dst, p, tbs):
                """Split a halfpair projection into KD per-kd closures (the
                last also copies out) so it can drip into the PE stream."""
                acc = {}

                def unit(kd):
                    def run():
                        if kd == 0:
                            acc['t'] = [auxps.tile(
                                [P, TQ], F32, tag="aux",
                                name=f"pj_{id(ws)}_{p}_{tb}") for tb in tbs]
                        for i, tb in enumerate(tbs):
                            nc.tensor.matmul(
                                acc['t'][i][:],
                                ws[:, kd * 256 + p * P: kd * 256 + (p + 1) * P],
                                xt[:, kd * T + tb * TQ: kd * T + (tb + 1) * TQ],
                                start=(kd == 0), stop=(kd == KD - 1))
                        if kd == KD - 1:
                            for i, tb in enumerate(tbs):
                                nc.vector.tensor_copy(
                                    dst[:, p * T + tb * TQ:
                                        p * T + (tb + 1) * TQ],
                                    acc['t'][i][:])
                    return run
                return [unit(kd) for kd in range(KD)]

            def emit_v_chunk(tci):
                pv = auxps.tile([P, 256], F32, tag="aux", name=f"pv_{tci}")
                for kd in range(KD):
                    nc.tensor.matmul(
                        pv[:],
                        xt[:, kd * T + tci * P: kd * T + (tci + 1) * P],
                        wvs[:, kd * 256:(kd + 1) * 256],
                        start=(kd == 0), stop=(kd == KD - 1))
                dst = v_sb[:, tci * VSLOT: (tci + 1) * VSLOT]
                nc.vector.tensor_copy(
                    dst.rearrange("p (h c) -> p h c", c=VW)[:, :, 0:DH],
                    pv.rearrange("p (h c) -> p h c", c=DH))

            # drip-feed queue: V chunks land in pair 0 / qb 0's stream
            # (one per key chunk, just ahead of the PV that consumes it),
            # then pair-1 projections through the rest of pair 0
            drip = [(lambda t: (lambda: emit_v_chunk(t)))(tci)
                    for tci in range(NT)]
            for args in ((wks, kt, 1, [0, 1]), (wks, kt, 1, [2, 3]),
                         (wqs, qt, 1, [0, 1]), (wqs, qt, 1, [2, 3])):
                drip.extend(proj_units(*args))

            def oproj_unit(qb, tc_):
                def run():
                    pys = [auxps.tile([P, TQ], F32, tag="aux",
                                      name=f"py{qb}_{tc_}_{n}")
                           for n in range(2)]
                    for p in range(NPAIR):
                        for nh in range(2):
                            nc.tensor.matmul(
                                pys[nh][:],
                                ot[:, p * T + qb * TQ + tc_ * P:
                                   p * T + qb * TQ + (tc_ + 1) * P],
                                wos[:, p * D + nh * TQ: p * D + (nh + 1) * TQ],
                                start=(p == 0), stop=(p == NPAIR - 1))
                    for nh in range(2):
                        ys = ysp.tile([P, TQ], F16, tag="ys")
                        nc.vector.tensor_copy(ys[:], pys[nh][:])
                        nc.sync.dma_start(
                            yp[qb * TQ + tc_ * P: qb * TQ + (tc_ + 1) * P,
                               nh * TQ:(nh + 1) * TQ],
                            ys[:])
                return run

            def normalize(p, qb, pva, pvb):
                for hi, pv_ in ((0, pva), (1, pvb)):
                    pvs = rcp.tile([VW, TQ], F32, tag="pvs")
                    nc.vector.tensor_copy(pvs[:], pv_[:])
                    sr = rcp.tile([1, TQ], F32, tag="sr")
                    nc.vector.tensor_copy(sr[0:1, :], pvs[DH:DH + 1, :])
                    rc = rcp.tile([1, TQ], F32, tag="rc")
                    nc.vector.reciprocal_approx_fast(rc[0:1, :], sr[0:1, :])
                    rb = rbp.tile([P, TQ], F32, tag="rb")
                    nc.gpsimd.partition_broadcast(rb[0:DH, :], rc[0:1, :],
                                                  channels=DH)
                    if hi == 0:
                        nc.vector.tensor_mul(
                            ot[0:DH, p * T + qb * TQ: p * T + (qb + 1) * TQ],
                            pvs[0:DH, :], rb[0:DH, :])
                    else:
                        sh = rbp.tile([P, TQ], F16, tag="sh")
                        nc.vector.tensor_mul(
                            sh[0:DH, :], pvs[0:DH, :], rb[0:DH, :])
                        nc.sync.dma_start(
                            ot[DH:P, p * T + qb * TQ: p * T + (qb + 1) * TQ],
                            sh[0:DH, :])

            def emit_pv(p, kc, pt_, pva, pvb):
                for hh, pv_ in ((0, pva), (1, pvb)):
                    h = 2 * p + hh
                    va = v_sb[:, kc * VSLOT + h * VW:
                              kc * VSLOT + h * VW + VW]
                    nc.tensor.matmul(
                        pv_[:], va, pt_[:, hh * TQ:(hh + 1) * TQ],
                        start=(kc == 0), stop=(kc == NT - 1))

            # ---- attention: pair-outer, query-block inner ----
            for p in range(NPAIR):
                for qb in range(NQ):
                    pva = pvps.tile([VW, TQ], F32, tag="pv")
                    pvb = pvps.tile([VW, TQ], F32, tag="pv")
                    pend = None  # PV deferred by one kc
                    for kc in range(NT):
                        lg = lgps.tile([P, 2 * TQ], F32, tag="lg")
                        for hh in range(2):
                            nc.tensor.matmul(
                                lg[:, hh * TQ:(hh + 1) * TQ],
                                kt[hh * DH:(hh + 1) * DH,
                                   p * T + kc * P: p * T + (kc + 1) * P],
                                qt[hh * DH:(hh + 1) * DH,
                                   p * T + qb * TQ: p * T + (qb + 1) * TQ],
                                start=True, stop=True)
                        # drip one queued unit (V chunk, pair-1 projection
                        # kd-step, or o-proj token chunk) into the PE stream
                        if kc >= 1 and drip:
                            drip.pop(0)()
                        pt_ = ptp.tile([P, 2 * TQ], F16, tag="pt")
                        nc.scalar.activation(pt_[:], lg[:], EXPF, scale=0.125)
                        if pend is not None:
                            emit_pv(p, pend[0], pend[1], pva, pvb)
                        pend = (kc, pt_)
                    if drip:
                        drip.pop(0)()
                    emit_pv(p, pend[0], pend[1], pva, pvb)
                    normalize(p, qb, pva, pvb)
                    if p == 1:
                        # o-proj for this block drips into the next block's
                        # key-chunk stream; the last block's runs at the tail
                        drip.extend(oproj_unit(qb, tc_) for tc_ in range(NQ))
            while drip:
                drip.pop(0)()

    nc.compile()
    return nc


_NC_CACHE = None


def _get_nc():
    global _NC_CACHE
    if _NC_CACHE is None:
        _NC_CACHE = build_nc()
    return _NC_CACHE


def _shard_inputs(x, Wqkv, Wo):
    x16 = np.asarray(x, dtype=np.float32).astype(np.float16)
    w16 = np.asarray(Wqkv, dtype=np.float32).astype(np.float16)
    wo16 = np.asarray(Wo, dtype=np.float32).astype(np.float16)
    xts = [np.ascontiguousarray(x16[b].T) for b in range(B)]
    in_maps = []
    for c in range(N_CORES):
        b, hq = c // 4, c % 4
        cs = hq * NH * DH
        in_maps.append({
            "xt_in": xts[b],
            "wq": np.ascontiguousarray(w16[:, cs:cs + NH * DH]),
            "wk": np.ascontiguousarray(w16[:, D + cs: D + cs + NH * DH]),
            "wv": np.ascontiguousarray(w16[:, 2 * D + cs: 2 * D + cs + NH * DH]),
            "wo": np.ascontiguousarray(wo16[cs:cs + NH * DH, :]),
        })
    return in_maps


def kernel(x, Wqkv, Wo):
    from concourse.bass_utils import run_bass_kernel_spmd

    nc = _get_nc()
    in_maps = _shard_inputs(x, Wqkv, Wo)
    res = run_bass_kernel_spmd(nc, in_maps, core_ids=list(range(N_CORES)))
    out = np.zeros((B, T, D), dtype=np.float32)
    for c in range(N_CORES):
        b = c // 4
        out[b] += res.results[c]["yp"].astype(np.float32)
    return out


# revision 19
# speedup vs baseline: 1.1888x; 1.0326x over previous
"""Multi-head attention (B=2, T=2048, D=1024, H=16, no causal mask) on 8 trn2
NeuronCores.

Sharding: (batch x head-quad).  Core c handles batch b = c // 4 and heads
[4*hq, 4*hq+4) with hq = c % 4.  Each core computes Q/K/V for only its 4
heads over the full 2048 tokens of its batch (no redundant projection work),
runs full attention for those heads, and emits a PARTIAL o-proj
y_c = out_heads @ Wo[head_rows] as fp16.  The host sums the 4 partials per
batch (the unshard step) - no on-device collective (collective_compute has an
~80us floor in this environment).

Host-side prep: X[b] is transposed on the host so the kernel runs zero PE
transposes; weight slices are pre-cut per core.

Per-core schedule (pair p holds heads 2p/2p+1 on partitions 0-63 / 64-127
of qt/kt row chunks; ScalarE exp at ~1.1us per [128,1024] tile is the pacing
engine at ~142us total):
  prelude: KT/QT for pair 0 (kd-outer, 2 open accumulators to amortize
           LDWEIGHTS), then V chunks in token order.
  attention (pair-outer, query-block inner, 16 key chunks):
      lg[k, q] = KT_h^T @ QT_h   (PSUM [128, 2*512]; hh row-tiles run
                                  concurrently on the PE)
      PT = exp(0.125 * lg)       (ScalarE -> fp16 SBUF)
      PV(kc-1) is emitted AFTER logits(kc) so the in-order TensorE stream
      never blocks waiting on exp; KT/QT for pair 1 are drip-fed into the
      PE slack during pair 0's attention, o-proj for block qb into pair 1's
      (qb+1) stream.
  o-proj: y_partial[q,:] = ot^T @ Wo_p accumulated over both pairs.
"""

import numpy as np

import concourse.bacc as bacc
import concourse.mybir as mybir
import concourse.tile as tile

F32 = mybir.dt.float32
F16 = mybir.dt.float16

B, T, D, H = 2, 2048, 1024, 16
DH = D // H          # 64
P = 128
N_CORES = 8
NH = 4               # heads per core
NPAIR = 2            # head pairs per core
KD = D // P          # 8 contraction chunks over D
NT = T // P          # 16 key-token chunks
TQ = 512             # query block
NQ = T // TQ         # 4 query blocks
VW = DH + 1          # 65: V head slot width incl. ones column
VSLOT = NH * VW      # 260 per key chunk
EXPF = mybir.ActivationFunctionType.Exp


def build_nc():
    nc = bacc.Bacc("TRN2", target_bir_lowering=False, debug=False,
                   num_devices=N_CORES)
    xt_in = nc.dram_tensor("xt_in", [D, T], F16, kind="ExternalInput").ap()
    wq = nc.dram_tensor("wq", [D, NH * DH], F16, kind="ExternalInput").ap()
    wk = nc.dram_tensor("wk", [D, NH * DH], F16, kind="ExternalInput").ap()
    wv = nc.dram_tensor("wv", [D, NH * DH], F16, kind="ExternalInput").ap()
    wo = nc.dram_tensor("wo", [NH * DH, D], F16, kind="ExternalInput").ap()
    yp = nc.dram_tensor("yp", [T, D], F16, kind="ExternalOutput").ap()

    wq3 = wq.rearrange("(kd p) c -> p kd c", p=P)
    wk3 = wk.rearrange("(kd p) c -> p kd c", p=P)
    wv3 = wv.rearrange("(kd p) c -> p kd c", p=P)
    xt3 = xt_in.rearrange("(kd p) t -> p kd t", p=P)

    with tile.TileContext(nc) as tc:
      with tc.tile_pool(name="persist", bufs=1) as persist:
        xt = persist.tile([P, KD * T], F16)       # 32 KB/part
        qt = persist.tile([P, NPAIR * T], F16)    # 8 KB/part
        kt = persist.tile([P, NPAIR * T], F16)    # 8 KB/part
        v_sb = persist.tile([P, NT * VSLOT], F16)  # 8.1 KB/part
        wqs = persist.tile([P, KD * 256], F16)    # 4 KB/part
        wks = persist.tile([P, KD * 256], F16)
        wvs = persist.tile([P, KD * 256], F16)
        wos = persist.tile([P, NPAIR * D], F16)   # 4 KB/part
        ot = persist.tile([P, NPAIR * T], F16)    # 8 KB/part

        # K/Q weights and xt chunks on the sync queue (needed first);
        # V/O weights on the gpsimd queue so they don't delay xt
        nc.sync.dma_start(wks.rearrange("p (kd c) -> p kd c", c=256), wk3)
        nc.sync.dma_start(wqs.rearrange("p (kd c) -> p kd c", c=256), wq3)
        for kd in range(KD):
            nc.sync.dma_start(xt[:, kd * T:(kd + 1) * T], xt3[:, kd])
        nc.gpsimd.dma_start(wvs.rearrange("p (kd c) -> p kd c", c=256), wv3)
        nc.gpsimd.dma_start(wos.rearrange("p (pp c) -> p pp c", c=D),
                            wo.rearrange("(pp r) c -> r pp c", r=P))

        onec = persist.tile([P, 1], F16)
        nc.vector.memset(onec[:], 1.0)
        nc.vector.tensor_copy(
            v_sb.rearrange("p (s c) -> p s c", c=VW)[:, :, DH:DH + 1],
            onec.unsqueeze(1).broadcast_to((P, NT * NH, 1)))

        # PE warmup: keep the tensor engine from HAM-throttling to 50%
        # during the input-DMA wait (zeros in/out, result discarded)
        warm = persist.tile([P, 256], F16)
        nc.vector.memset(warm[:], 0.0)

        with tc.tile_pool(name="warmps", bufs=1, space="PSUM") as warmps:
            wps = warmps.tile([P, 256], F32)
            for _ in range(36):
                nc.tensor.matmul(wps[:], warm[:, 0:P], warm[:],
                                 start=True, stop=True)

        with tc.tile_pool(name="preps", bufs=8, space="PSUM") as preps:
            accs = {}
            for hp, (ws, dst, tbs) in enumerate(
                    ((wks, kt, (0, 1)), (wks, kt, (2, 3)),
                     (wqs, qt, (0, 1)), (wqs, qt, (2, 3)))):
                accs[hp] = [preps.tile([P, TQ], F32, tag="pre",
                                       name=f"pre_{hp}_{tb}") for tb in tbs]
            for kd in range(KD):
                for hp, (ws, dst, tbs) in enumerate(
                        ((wks, kt, (0, 1)), (wks, kt, (2, 3)),
                         (wqs, qt, (0, 1)), (wqs, qt, (2, 3)))):
                    for i, tb in enumerate(tbs):
                        nc.tensor.matmul(
                            accs[hp][i][:],
                            ws[:, kd * 256: kd * 256 + P],
                            xt[:, kd * T + tb * TQ: kd * T + (tb + 1) * TQ],
                            start=(kd == 0), stop=(kd == KD - 1))
            for hp, (ws, dst, tbs) in enumerate(
                    ((wks, kt, (0, 1)), (wks, kt, (2, 3)),
                     (wqs, qt, (0, 1)), (wqs, qt, (2, 3)))):
                for i, tb in enumerate(tbs):
                    nc.vector.tensor_copy(
                        dst[:, tb * TQ:(tb + 1) * TQ], accs[hp][i][:])

        with (
            tc.tile_pool(name="auxps", bufs=2, space="PSUM") as auxps,
            tc.tile_pool(name="lgps", bufs=2, space="PSUM") as lgps,
            tc.tile_pool(name="pvps", bufs=2, space="PSUM") as pvps,
            tc.tile_pool(name="ptp", bufs=4) as ptp,
            tc.tile_pool(name="rcp", bufs=2) as rcp,
            tc.tile_pool(name="rbp", bufs=3) as rbp,
            tc.tile_pool(name="ysp", bufs=3) as ysp,
        ):
            # ---- projection emitters (used for prelude AND drip-feed) ----
            def emit_proj_halfpair(ws, dst, p, tbs):
                """KT or QT row chunk p for query blocks tbs (kd-outer,
                len(tbs) open accumulators)."""
                for u in proj_units(ws, dst, p, tbs):
                    u()

            def proj_units(ws, dst, p, tbs):
                """Split a halfpair projection into KD per-kd closures (the
                last also copies out) so it can drip into the PE stream."""
                acc = {}

                def unit(kd):
                    def run():
                        if kd == 0:
                            acc['t'] = [auxps.tile(
                                [P, TQ], F32, tag="aux",
                                name=f"pj_{id(ws)}_{p}_{tb}") for tb in tbs]
                        for i, tb in enumerate(tbs):
                            nc.tensor.matmul(
                                acc['t'][i][:],
                                ws[:, kd * 256 + p * P: kd * 256 + (p + 1) * P],
                                xt[:, kd * T + tb * TQ: kd * T + (tb + 1) * TQ],
                                start=(kd == 0), stop=(kd == KD - 1))
                        if kd == KD - 1:
                            for i, tb in enumerate(tbs):
                                nc.vector.tensor_copy(
                                    dst[:, p * T + tb * TQ:
                                        p * T + (tb + 1) * TQ],
                                    acc['t'][i][:])
                    return run
                return [unit(kd) for kd in range(KD)]

            def emit_v_chunk(tci):
                pv = auxps.tile([P, 256], F32, tag="aux", name=f"pv_{tci}")
                for kd in range(KD):
                    nc.tensor.matmul(
                        pv[:],
                        xt[:, kd * T + tci * P: kd * T + (tci + 1) * P],
                        wvs[:, kd * 256:(kd + 1) * 256],
                        start=(kd == 0), stop=(kd == KD - 1))
                dst = v_sb[:, tci * VSLOT: (tci + 1) * VSLOT]
                nc.vector.tensor_copy(
                    dst.rearrange("p (h c) -> p h c", c=VW)[:, :, 0:DH],
                    pv.rearrange("p (h c) -> p h c", c=DH))

            # drip-feed queue: V chunks land in pair 0 / qb 0's stream
            # (one per key chunk, just ahead of the PV that consumes it),
            # then pair-1 projections through the rest of pair 0
            drip = [(lambda t: (lambda: emit_v_chunk(t)))(tci)
                    for tci in range(NT)]
            for args in ((wks, kt, 1, [0, 1]), (wks, kt, 1, [2, 3]),
                         (wqs, qt, 1, [0, 1]), (wqs, qt, 1, [2, 3])):
                drip.extend(proj_units(*args))

            def oproj_unit(qb, tc_):
                def run():
                    pys = [auxps.tile([P, TQ], F32, tag="aux",
                                      name=f"py{qb}_{tc_}_{n}")
                           for n in range(2)]
                    for p in range(NPAIR):
                        for nh in range(2):
                            nc.tensor.matmul(
                                pys[nh][:],
                                ot[:, p * T + qb * TQ + tc_ * P:
                                   p * T + qb * TQ + (tc_ + 1) * P],
                                wos[:, p * D + nh * TQ: p * D + (nh + 1) * TQ],
                                start=(p == 0), stop=(p == NPAIR - 1))
                    for nh in range(2):
                        ys = ysp.tile([P, TQ], F16, tag="ys")
                        nc.vector.tensor_copy(ys[:], pys[nh][:])
                        nc.sync.dma_start(
                            yp[qb * TQ + tc_ * P: qb * TQ + (tc_ + 1) * P,
                               nh * TQ:(nh + 1) * TQ],
                            ys[:])
                return run

            def normalize(p, qb, pva, pvb):
                for hi, pv_ in ((0, pva), (1, pvb)):
                    pvs = rcp.tile([VW, TQ], F32, tag="pvs")
                    nc.vector.tensor_copy(pvs[:], pv_[:])
                    sr = rcp.tile([1, TQ], F32, tag="sr")
                    nc.vector.tensor_copy(sr[0:1, :], pvs[DH:DH + 1, :])
                    rc = rcp.tile([1, TQ], F32, tag="rc")
                    nc.vector.reciprocal_approx_fast(rc[0:1, :], sr[0:1, :])
                    rb = rbp.tile([P, TQ], F32, tag="rb")
                    nc.gpsimd.partition_broadcast(rb[0:DH, :], rc[0:1, :],
                                                  channels=DH)
                    if hi == 0:
                        nc.vector.tensor_mul(
                            ot[0:DH, p * T + qb * TQ: p * T + (qb + 1) * TQ],
                            pvs[0:DH, :], rb[0:DH, :])
                    else:
                        sh = rbp.tile([P, TQ], F16, tag="sh")
                        nc.vector.tensor_mul(
                            sh[0:DH, :], pvs[0:DH, :], rb[0:DH, :])
                        nc.sync.dma_start(
                            ot[DH:P, p * T + qb * TQ: p * T + (qb + 1) * TQ],
                            sh[0:DH, :])

            def emit_pv(p, kc, pt_, pva, pvb):
                for hh, pv_ in ((0, pva), (1, pvb)):
                    h = 2 * p + hh
                    va = v_sb[:, kc * VSLOT + h * VW:
                              kc * VSLOT + h * VW + VW]
                    nc.tensor.matmul(
                        pv_[:], va, pt_[:, hh * TQ:(hh + 1) * TQ],
                        start=(kc == 0), stop=(kc == NT - 1))

            # ---- attention: pair-outer, query-block inner ----
            for p in range(NPAIR):
                for qb in range(NQ):
                    pva = pvps.tile([VW, TQ], F32, tag="pv")
                    pvb = pvps.tile([VW, TQ], F32, tag="pv")
                    pend = None  # PV deferred by one kc
                    for kc in range(NT):
                        lg = lgps.tile([P, 2 * TQ], F32, tag="lg")
                        for hh in range(2):
                            nc.tensor.matmul(
                                lg[:, hh * TQ:(hh + 1) * TQ],
                                kt[hh * DH:(hh + 1) * DH,
                                   p * T + kc * P: p * T + (kc + 1) * P],
                                qt[hh * DH:(hh + 1) * DH,
                                   p * T + qb * TQ: p * T + (qb + 1) * TQ],
                                start=True, stop=True)
                        # drip one queued unit (V chunk, pair-1 projection
                        # kd-step, or o-proj token chunk) into the PE stream
                        if kc >= 1 and drip:
                            drip.pop(0)()
                        pt_ = ptp.tile([P, 2 * TQ], F16, tag="pt")
                        nc.scalar.activation(pt_[:], lg[:], EXPF, scale=0.125)
                        if pend is not None:
                            emit_pv(p, pend[0], pend[1], pva, pvb)
                        pend = (kc, pt_)
                    if drip:
                        drip.pop(0)()
                    emit_pv(p, pend[0], pend[1], pva, pvb)
                    normalize(p, qb, pva, pvb)
                    if p == 1:
                        # o-proj for this block drips into the next block's
                        # key-chunk stream; the last block's runs at the tail
                        drip.extend(oproj_unit(qb, tc_) for tc_ in range(NQ))
            while drip:
                drip.pop(0)()

    nc.compile()
    return nc


_NC_CACHE = None


def _get_nc():
    global _NC_CACHE
    if _NC_CACHE is None:
        _NC_CACHE = build_nc()
    return _NC_CACHE


def _shard_inputs(x, Wqkv, Wo):
    x16 = np.asarray(x, dtype=np.float32).astype(np.float16)
    w16 = np.asarray(Wqkv, dtype=np.float32).astype(np.float16)
    wo16 = np.asarray(Wo, dtype=np.float32).astype(np.float16)
    xts = [np.ascontiguousarray(x16[b].T) for b in range(B)]
    in_maps = []
    for c in range(N_CORES):
        b, hq = c // 4, c % 4
        cs = hq * NH * DH
        in_maps.append({
            "xt_in": xts[b],
            "wq": np.ascontiguousarray(w16[:, cs:cs + NH * DH]),
            "wk": np.ascontiguousarray(w16[:, D + cs: D + cs + NH * DH]),
            "wv": np.ascontiguousarray(w16[:, 2 * D + cs: 2 * D + cs + NH * DH]),
            "wo": np.ascontiguousarray(wo16[cs:cs + NH * DH, :]),
        })
    return in_maps


def kernel(x, Wqkv, Wo):
    from concourse.bass_utils import run_bass_kernel_spmd

    nc = _get_nc()
    in_maps = _shard_inputs(x, Wqkv, Wo)
    res = run_bass_kernel_spmd(nc, in_maps, core_ids=list(range(N_CORES)))
    out = np.zeros((B, T, D), dtype=np.float32)
    for c in range(N_CORES):
        b = c // 4
        out[b] += res.results[c]["yp"].astype(np.float32)
    return out
